# revision 11
# baseline (speedup 1.0000x reference)
"""Distributed brute-force KNN kernel for one TRN2 chip (8 NeuronCores).

Problem: queries [256,128] f32, candidates [500000,128] f32, identifiers
[500000] i32, k=100. Output: (values [256,100] f32 desc, ids [256,100] i32).

Device strategy (per core, candidates sharded N/8 = 62500, padded 64512):
  - bf16 matmul in 1024-col "quanta" (one query half each) -> PSUM f32
    (2 banks per quantum, 4 quanta in flight).
  - Per batch of 7 blocks: first NDB=2 blocks are "D" role -> DVE
    tensor_reduce(max) folds the psum quantum 1024->128 (FOLD=8) straight
    into the slot accumulator (slot j covers candidates 8j..8j+7).
    Remaining 5 are "A" role -> ScalarE copies psum -> bf16 staging; a
    batched DVE tensor_max chain folds 1024->512->256->128 (slot j covers
    j+128m, m<8). Roles keep DVE and Act balanced (~0.76 ns/score).
  - Slot maxima [128, 2*8064] bf16 DMA'd out; no top-k on device.
Host: threshold the slot stream, expand + rescore candidate groups exactly
in f64, iterate until provably complete, emit exact top-k (value desc,
index asc tiebreak). Exactness never depends on device numerics.
"""
import numpy as np
import ml_dtypes

B = 256          # queries
N = 500000       # candidates
D = 128          # dim
NCORES = 8
NSH = N // NCORES            # 62500 real candidates per core
QCOLS = 1024                 # candidate cols per matmul quantum
NBLOCKS = 63                 # 1024-col blocks per core
NSHP = NBLOCKS * QCOLS       # 64512 padded
FOLD = 8                     # candidates per slot
S = QCOLS // FOLD            # 128 slots per block
NSLOTS = NBLOCKS * S         # 8064 slots per (core, query)
BATCH = 7                    # blocks per chain flush
NDB = 2                      # leading "D"-role blocks per batch

_CACHE = {}


def block_role(blk):
    """'D' (tensor_reduce direct) or 'A' (act copy + chain) for a block."""
    return "D" if (blk % BATCH) < NDB else "A"


def build(loops=1, nblocks=NBLOCKS, batch=BATCH, ndb=NDB, chunk=BATCH,
          variant="fp8"):
    """Build + compile the per-core Bass program.

    variant="fp8": e4m3 inputs in DoubleRow plane layout [64, 2, cols]
    (2x PE rate, half the DMA bytes). variant None/"bf16": bf16 inputs.
    """
    import concourse.bass as bass
    import concourse.tile as tile
    from concourse import bacc, mybir

    bf16 = mybir.dt.bfloat16
    f32 = mybir.dt.float32
    Copy = mybir.ActivationFunctionType.Copy
    X = mybir.AxisListType.X
    MAX = mybir.AluOpType.max
    fp8 = variant == "fp8"

    nshp = nblocks * QCOLS
    nslots = nblocks * S
    nab = batch - ndb
    assert nblocks % batch == 0 and batch % chunk == 0

    nc = bacc.Bacc("TRN2", debug=False)
    if fp8:
        e4 = mybir.dt.float8e4
        DR = mybir.MatmulPerfMode.DoubleRow
        qt = nc.dram_tensor("qt", [64, 2, B], e4, kind="ExternalInput").ap()
        ct = nc.dram_tensor("ct", [64, 2, nshp], e4, kind="ExternalInput").ap()
    else:
        qt = nc.dram_tensor("qt", [D, B], bf16, kind="ExternalInput").ap()
        ct = nc.dram_tensor("ct", [D, nshp], bf16, kind="ExternalInput").ap()
    sv = nc.dram_tensor("sv", [128, 2 * nslots], bf16, kind="ExternalOutput").ap()

    with tile.TileContext(nc) as tc:
        with (
            tc.tile_pool(name="qpool", bufs=1) as qpool,
            tc.tile_pool(name="cpool", bufs=2) as cpool,
            tc.tile_pool(name="psum", bufs=4, space="PSUM") as pp,
            tc.tile_pool(name="stage", bufs=2) as sp,
            tc.tile_pool(name="acc", bufs=1) as accp,
        ):
            if fp8:
                qtile = qpool.tile([64, 2, B], mybir.dt.float8e4)
            else:
                qtile = qpool.tile([D, B], bf16)
            nc.sync.dma_start(qtile[:], qt[:])
            vacc = accp.tile([128, 2 * nslots], bf16, tag="vacc", name="vacc")
            v4 = vacc.rearrange("p (h b s) -> p h b s", h=2, s=S)

            def body(_iv=None):
                for b0 in range(0, nblocks, batch):
                    bA = (
                        sp.tile([128, 2 * nab, 1024], bf16, tag="bA", name="bA")
                        if nab
                        else None
                    )
                    iA = 0
                    ctile = None
                    for bb in range(batch):
                        blk = b0 + bb
                        role = "D" if bb < ndb else "A"
                        if bb % chunk == 0:
                            if fp8:
                                ctile = cpool.tile(
                                    [64, 2, chunk * QCOLS], mybir.dt.float8e4,
                                    tag="ct", name="ctile",
                                )
                                nc.sync.dma_start(
                                    ctile[:],
                                    ct[:, :, bass.ds(blk * QCOLS, chunk * QCOLS)],
                                )
                            else:
                                ctile = cpool.tile(
                                    [D, chunk * QCOLS], bf16, tag="ct", name="ctile"
                                )
                                nc.sync.dma_start(
                                    ctile[:],
                                    ct[:, bass.ds(blk * QCOLS, chunk * QCOLS)],
                                )
                        off = (bb % chunk) * QCOLS
                        for h in range(2):
                            ps = pp.tile([128, QCOLS], f32, name="ps")
                            for mh in range(2):
                                if fp8:
                                    nc.tensor.matmul(
                                        ps[:, bass.ds(mh * 512, 512)],
                                        lhsT=qtile[:, :, bass.ds(h * 128, 128)],
                                        rhs=ctile[:, :, bass.ds(off + mh * 512, 512)],
                                        start=True,
                                        stop=True,
                                        perf_mode=DR,
                                    )
                                else:
                                    nc.tensor.matmul(
                                        ps[:, bass.ds(mh * 512, 512)],
                                        lhsT=qtile[:, bass.ds(h * 128, 128)],
                                        rhs=ctile[:, bass.ds(off + mh * 512, 512)],
                                        start=True,
                                        stop=True,
                                    )
                            if role == "D":
                                ps3 = ps.rearrange("p (g w) -> p g w", w=FOLD)
                                nc.vector.tensor_reduce(
                                    v4[:, h, blk, :], ps3[:], axis=X, op=MAX
                                )
                            else:
                                nc.scalar.activation(bA[:, iA, :], ps[:], Copy)
                                iA += 1
                    # batched bf16 chain for A-role quanta (order: blk-major,
                    # h-minor => stage row 2*(bb-ndb)+h)
                    if nab:
                        c1 = sp.tile([128, 2 * nab, 512], bf16, tag="c1", name="c1")
                        nc.vector.tensor_max(c1[:], bA[:, :, 0:512], bA[:, :, 512:1024])
                        w = sp.tile([128, 2 * nab, 256], bf16, tag="w", name="w")
                        nc.vector.tensor_max(w[:], c1[:, :, 0:256], c1[:, :, 256:512])
                        for h in range(2):
                            src = w[:, h: 2 * nab: 2, :]
                            dst = v4[:, h, bass.ds(b0 + ndb, nab), :]
                            nc.vector.tensor_max(
                                dst, src[:, :, 0:128], src[:, :, 128:256]
                            )
                    for h in range(2):
                        nc.sync.dma_start(
                            sv[:, bass.ds(h * nslots + b0 * S, batch * S)],
                            vacc[:, bass.ds(h * nslots + b0 * S, batch * S)],
                        )

            if loops == 1:
                body()
            else:
                with tc.For_i(0, loops, 1) as iv:
                    body(iv)
    nc.compile()
    return nc


def _get_nc():
    if "nc" not in _CACHE:
        _CACHE["nc"] = build()
    return _CACHE["nc"]


def make_in_maps(queries, candidates, variant="fp8"):
    if variant == "fp8":
        e4 = ml_dtypes.float8_e4m3
        q8 = queries.astype(e4)  # [B, D]
        qt = np.ascontiguousarray(q8.T.reshape(2, 64, B).transpose(1, 0, 2))
        cb = candidates.astype(e4)
        in_maps = []
        for c in range(NCORES):
            ct = np.zeros((64, 2, NSHP), dtype=e4)
            sh = cb[c * NSH: (c + 1) * NSH].T  # [D, NSH]
            ct[:, :, :NSH] = sh.reshape(2, 64, NSH).transpose(1, 0, 2)
            in_maps.append({"qt": qt, "ct": ct})
        return in_maps
    qt = np.ascontiguousarray(queries.T).astype(ml_dtypes.bfloat16)
    cb = candidates.astype(ml_dtypes.bfloat16)
    in_maps = []
    for c in range(NCORES):
        ct = np.zeros((D, NSHP), dtype=ml_dtypes.bfloat16)
        ct[:, :NSH] = cb[c * NSH: (c + 1) * NSH].T
        in_maps.append({"qt": qt, "ct": ct})
    return in_maps


def _device_slots(queries, candidates):
    """Run the 8-core SPMD kernel; return slot maxima [NCORES, B, NSLOTS] f32."""
    from concourse.bass_utils import run_bass_kernel_spmd

    nc = _get_nc()
    in_maps = make_in_maps(queries, candidates)
    res = None
    for attempt in range(3):
        try:
            res = run_bass_kernel_spmd(nc, in_maps, core_ids=list(range(NCORES))).results
            break
        except Exception:
            if attempt == 2:
                raise
            import time as _time
            _time.sleep(2.0)
    assert res is not None
    out = np.empty((NCORES, B, NSLOTS), np.float32)
    for c in range(NCORES):
        svc = np.asarray(res[c]["sv"]).astype(np.float32)
        out[c, :128] = svc[:, :NSLOTS]
        out[c, 128:] = svc[:, NSLOTS:]
    return out


# Slot membership depends on the block's role:
#   'D': slot j of block b -> local candidates b*QCOLS + 8*j + m, m<8
#   'A': slot j of block b -> local candidates b*QCOLS + j + 128*m, m<8
_ROLE_D = np.array([block_role(b) == "D" for b in range(NBLOCKS)])


def _slot_members(slot_ids):
    """Global slot ids [0, NCORES*NSLOTS) -> member candidate global indices
    [..., FOLD]; -1 where padded/invalid."""
    core = slot_ids // NSLOTS
    rem = slot_ids % NSLOTS
    blk = rem // S
    j = rem % S
    is_d = _ROLE_D[blk]
    m = np.arange(FOLD)
    mem_d = (blk * QCOLS + 8 * j)[..., None] + m[None, :]
    mem_a = (blk * QCOLS + j)[..., None] + (m * S)[None, :]
    mem = np.where(is_d[..., None], mem_d, mem_a)
    valid = mem < NSH
    gl = core[..., None] * NSH + np.minimum(mem, NSH - 1)
    return np.where(valid, gl, -1)


def kernel(queries, candidates, identifiers, k):
    queries = np.asarray(queries, dtype=np.float32)
    candidates = np.asarray(candidates, dtype=np.float32)
    identifiers = np.asarray(identifiers)
    kk = int(k)

    sv = _device_slots(queries, candidates)               # [8, B, NSLOTS]
    V = sv.transpose(1, 0, 2).reshape(B, NCORES * NSLOTS)
    TS = V.shape[1]
    q64 = queries.astype(np.float64)

    J0 = max(2 * kk, kk + 92)
    sel = np.argpartition(-V, J0, axis=1)[:, :J0]
    selmask = np.zeros((B, TS), bool)
    np.put_along_axis(selmask, sel, True, 1)

    pool_v = [None] * B
    pool_g = [None] * B
    gmax = np.full((B, TS), -np.inf, np.float32)

    def rescore(q, slots):
        mem = _slot_members(slots)
        valid = mem >= 0
        gl = np.where(valid, mem, 0)
        svx = candidates[gl.reshape(-1)].reshape(*gl.shape, D).astype(np.float64)
        sc = svx @ q64[q]
        sc = np.where(valid, sc, -np.inf)
        return sc, mem

    for q in range(B):
        sc, mem = rescore(q, sel[q])
        pool_v[q] = sc.ravel()
        pool_g[q] = mem.ravel()
        gmax[q, sel[q]] = sc.max(1)

    for _round in range(8):
        fin = np.isfinite(gmax) & selmask
        under = np.where(fin, gmax - np.where(fin, V, 0), 0.0)
        eps = max(float(under.max()), 0.0)
        margin = 4.0 * eps + 0.05
        vk = np.empty(B)
        for q in range(B):
            vk[q] = -np.partition(-pool_v[q], kk - 1)[kk - 1]
        need = (V >= (vk[:, None] - margin)) & ~selmask
        if not need.any():
            break
        for q in np.nonzero(need.any(1))[0]:
            slots = np.nonzero(need[q])[0]
            sc, mem = rescore(q, slots)
            pool_v[q] = np.concatenate([pool_v[q], sc.ravel()])
            pool_g[q] = np.concatenate([pool_g[q], mem.ravel()])
            gmax[q, slots] = sc.max(1)
        selmask |= need
    else:
        raise RuntimeError("slot rescoring did not converge")

    out_v = np.empty((B, kk), np.float32)
    out_g = np.empty((B, kk), np.int64)
    for q in range(B):
        keep = pool_g[q] >= 0
        g, first = np.unique(pool_g[q][keep], return_index=True)
        v = pool_v[q][keep][first].astype(np.float32)
        assert v.size >= kk
        order = np.lexsort((g, -v))[:kk]
        out_v[q] = v[order]
        out_g[q] = g[order]

    top_ids = identifiers[out_g]
    return out_v, top_ids
